# revision 15
# baseline (speedup 1.0000x reference)
"""Causal single-head attention (B=4, T=4096, D=1024, H=64) on 8 TRN2 cores.

Sharding: core c -> batch b=c//2, parity p=c%2. Each core computes attention
output for the 16 interleaved query tiles {128*(2i+p)} of its batch.  The
program is SPMD-uniform: per-core differences (which rows, causal masks) are
carried entirely in the input data (host-side slicing + mask tiles).

QKV matmuls run in bf16 (fp32 PSUM); exp(S) is written as fp8e4m3 and the
PV matmuls run fp8 with DoubleRow pairing (two key tiles per matmul).
Causal masking is additive on the scores in PSUM (0 / -1e4), before exp.

Per query-span j (own q-tiles 4j..4j+3, 512 cols) and key phase (own keys
first, partner keys after passB), chunks are exact-causal: slots 0..4j-1
full-width, then 4 "straddle" slots 4j+u covering span cols [128u:512) with
an additive mask on the first 128 cols.  Score groups (one ACT exp each):
  [full 2i | full 2i+1]  (1024 cols)  -> one DoubleRow PV over [0:512)
  [u0 | u1]              (896 cols)   -> DR PV over [128:512) + u0 rem [0:128)
  [u2 | u3]              (384 cols)   -> DR PV over [384:512) + u2 rem [256:384)

Emission order: passA (own x cols -> q,k_own,v_own) -> own-key attention
(PV partials parked in SBUF) -> passB (partner cols; DMA streamed early) ->
partner-key attention + combine + epilogue.
"""

import os
import re
import numpy as np

B, T, D, H = 4, 4096, 1024, 64
NT = T // 128          # 32 key tiles per batch
NOWN = NT // 2         # 16 query tiles per core
GW = 1024              # max score-group width (cols)

_PROG = None
LAST_EXEC_TIME_NS = None
LAST_RESULTS = None


def _patch_tile_drain():
    """Walrus in this container allows only one sync-wait on NO_STRUCT
    instructions; TileContext's tail drain carries one wait per DMA lane.
    Split it into one drain per outstanding proc."""
    import bass_rust
    import concourse.tile as tile

    if getattr(tile.TileContext, "_drain_patched", False):
        return

    def _drain_and_barrier(self, tick_clock, wait_clock):
        nc = self.nc
        gvec = tick_clock.global_clock
        ticks = eval(re.match(r"VectorClock\((\[.*\])\)", repr(gvec)).group(1))
        for pr, tk in enumerate(ticks):
            if tk > 0:
                vec = [0] * len(ticks)
                vec[pr] = tk
                d = nc.sync.drain()
                wait_clock.add_sem_waits(
                    d.ins,
                    bass_rust.ScopedClock({None: bass_rust.VectorClock(vec)}),
                )
        nc.sync.drain()
        nc.all_engine_barrier()
        assert self.sems is not None
        popped = nc._tile_sem_poison_stack.pop()
        assert popped is self._sem_poison
        nc.clear_and_free_semaphores(list(self.sems.allocated().values()))
        nc.all_engine_barrier()

    tile.TileContext._drain_and_barrier = _drain_and_barrier
    tile.TileContext._drain_patched = True


def _split_multi_waits(nc):
    """This walrus build allows at most one sync-wait per instruction.
    Hoist extra waits onto injected same-engine NOPs placed just before the
    owning instruction (same engine stream => identical semantics)."""
    import bass_rust

    for bb in nc.main_func.blocks:
        new_list = []
        for ins in bb.instructions:
            si = ins.sync_info
            if si is not None and si.on_wait and len(si.on_wait) > 1:
                waits = list(si.on_wait)
                for w in waits[:-1]:
                    nop = nc.engines[ins.engine].nop().ins
                    # remove the nop from wherever engine.nop() appended it
                    for bb2 in nc.main_func.blocks:
                        if nop in bb2.instructions:
                            bb2.instructions.remove(nop)
                            break
                    nop.sync_info = bass_rust.SyncInfo(on_wait=[w], on_update=[])
                    new_list.append(nop)
                si.on_wait = [waits[-1]]
            new_list.append(ins)
        bb.instructions[:] = new_list


def _span_groups(j):
    """Score/PV group plan for span j (same for own/partner phases).
    Returns list of groups; each group:
      dict(width, st=[(slot_off, c0, w, goff)], masks=[(goff, span_c)],
           pv_pairs=[(slot_off, goff, out_c0, w)], pv_rem=[(slot_off, goff,
           out_c0, w)])
    slot_off is within-phase slot (0..4j+3); span_c/out_c0 are span cols."""
    groups = []
    for i in range(2 * j):  # full pairs
        a, b = 2 * i, 2 * i + 1
        groups.append(dict(
            width=1024,
            st=[(a, 0, 512, 0), (b, 0, 512, 512)],
            masks=[],
            pv_pairs=[(a, 0, 0, 512)],
            pv_rem=[],
        ))
    # straddle A: u0 (512 @ c0=0) | u1 (384 @ c0=128)
    u0, u1 = 4 * j, 4 * j + 1
    groups.append(dict(
        width=896,
        st=[(u0, 0, 512, 0), (u1, 128, 384, 512)],
        masks=[(0, 0), (512, 128)],       # u0 head at goff 0, u1 head at 512
        pv_pairs=[(u0, 128, 128, 384)],   # over span cols [128:512)
        pv_rem=[(u0, 0, 0, 128)],
    ))
    # straddle B: u2 (256 @ c0=256) | u3 (128 @ c0=384)
    u2, u3 = 4 * j + 2, 4 * j + 3
    groups.append(dict(
        width=384,
        st=[(u2, 256, 256, 0), (u3, 384, 128, 256)],
        masks=[(0, 256), (256, 384)],
        pv_pairs=[(u2, 128, 384, 128)],   # over span cols [384:512)
        pv_rem=[(u2, 0, 256, 128)],
    ))
    return groups


def _build_program():
    import concourse.bass as bass
    import concourse.tile as tile
    from concourse import mybir
    from concourse.masks import make_identity

    _patch_tile_drain()
    f32 = mybir.dt.float32
    bf16 = mybir.dt.bfloat16
    use_pair = os.environ.get("K_PAIR", "0") == "1"
    use_fp8 = os.environ.get("K_FP8", "0") == "1"
    fp8 = mybir.dt.float8e4 if use_fp8 else mybir.dt.bfloat16
    DR = mybir.MatmulPerfMode.DoubleRow

    nc = bass.Bass()
    xT = nc.dram_tensor("xT", [D, T], bf16, kind="ExternalInput")
    wqk = nc.dram_tensor("wqk", [128, D], bf16, kind="ExternalInput")
    wkv = nc.dram_tensor("wkv", [128, D], bf16, kind="ExternalInput")
    wv = nc.dram_tensor("wv", [128, D // 128 * H], bf16, kind="ExternalInput")
    masks = nc.dram_tensor("masks", [128, 2 * 128], f32, kind="ExternalInput")
    out = nc.dram_tensor("out", [128, NOWN * H], f32, kind="ExternalOutput")

    ND = D // 128  # 8 d-tiles

    with tile.TileContext(nc) as tc:
        with (
            tc.tile_pool(name="singles", bufs=1) as singles,
            tc.tile_pool(name="xta", bufs=8) as xta,
            tc.tile_pool(name="xtb", bufs=8) as xtb,
            tc.tile_pool(name="pp", bufs=3) as ppool,
            tc.tile_pool(name="op", bufs=2) as opool,
        ):
            # x tiles stream on the sync HWDGE queue (first need gates PE start)
            xa = []
            for d in range(ND):
                t_ = xta.tile([128, T // 2], bf16, tag="xa")
                nc.sync.dma_start(out=t_, in_=xT[d * 128 : (d + 1) * 128, 0 : T // 2])
                xa.append(t_)
            xb = []
            for d in range(ND):
                t_ = xtb.tile([128, T // 2], bf16, tag="xb")
                nc.sync.dma_start(out=t_, in_=xT[d * 128 : (d + 1) * 128, T // 2 : T])
                xb.append(t_)

            # constants go via gpsimd SWDGE so they don't delay the x stream
            wqk_sb = singles.tile([128, ND, 128], bf16)
            nc.gpsimd.dma_start(out=wqk_sb, in_=wqk.rearrange("p (dt h) -> p dt h", dt=ND))
            wv_sb = singles.tile([128, ND, H], bf16)
            nc.gpsimd.dma_start(out=wv_sb, in_=wv.rearrange("p (dt h) -> p dt h", dt=ND))
            mask_sb = singles.tile([128, 2, 128], f32)
            nc.gpsimd.dma_start(out=mask_sb, in_=masks.rearrange("p (m f) -> p m f", m=2))
            wkv_sb = singles.tile([128, ND, 128], bf16)
            nc.gpsimd.dma_start(out=wkv_sb, in_=wkv.rearrange("p (dt h) -> p dt h", dt=ND))
            ident_b = singles.tile([128, 128], bf16)
            make_identity(nc, ident_b)

            qT = singles.tile([64, T // 2], bf16)          # q^T own rows
            kT = singles.tile([64, T], bf16)               # k^T own-first layout
            vT_own = singles.tile([64, T // 2], bf16)
            vT_oth = singles.tile([64, T // 2], bf16)
            v_sb = singles.tile([128, NT, 80], fp8)        # v natural + ones col (pad: DR step%16)
            o_own = singles.tile([65, 4, 512], f32)        # own-key PV partials
            out_sb = singles.tile([128, NOWN, H], f32)

            nc.vector.memset(v_sb[:, :, H : H + 1], 1.0)
            expb = singles.tile([128, 1], f32)
            nc.vector.memset(expb, -2.0)

            # ---- pass A: [Wq|Wk] and Wv over own columns ----
            with tc.tile_pool(name="psA", bufs=1, space="PSUM") as psA:
                qk_ps = psA.tile([128, T // 2], f32, tag="qk")
                v_ps = psA.tile([64, T // 2], f32, tag="vp")
                for d in range(ND):
                    for tck in range(4):
                        sl = slice(tck * 512, (tck + 1) * 512)
                        nc.tensor.matmul(qk_ps[:, sl], lhsT=wqk_sb[:, d, :], rhs=xa[d][:, sl],
                                         start=(d == 0), stop=(d == ND - 1))
                        nc.tensor.matmul(v_ps[:, sl], lhsT=wv_sb[:, d, :], rhs=xa[d][:, sl],
                                         start=(d == 0), stop=(d == ND - 1))
                nc.vector.tensor_copy(out=kT[:, 0 : T // 2], in_=qk_ps[64:128, :])
                nc.vector.tensor_copy(out=qT, in_=qk_ps[0:64, :])
                nc.vector.tensor_copy(out=vT_own, in_=v_ps[:, :])

            # ---- attention + passB ----
            with tc.tile_pool(name="psB", bufs=1, space="PSUM") as psB:
                # v_own^T -> natural fp8 layout (slots 0..15)
                for i in range(NOWN):
                    tp = psB.tile([128, 65], bf16, tag="otp", bufs=1)
                    nc.tensor.transpose(tp[:, 0:H], vT_own[:, i * 128 : (i + 1) * 128],
                                        ident_b[0:64, 0:64])
                    nc.vector.tensor_copy(out=v_sb[:, i, 0:H], in_=tp[:, 0:H])

                def emit_phase(partner, epilogue):
                    base = 16 if partner else 0
                    mi = 1 if partner else 0
                    stream = []  # (j, group, first, last)
                    for j in range(4):
                        gs = _span_groups(j)
                        for gi, g in enumerate(gs):
                            stream.append((j, g, gi == 0, gi == len(gs) - 1))

                    sc_tiles = [None] * len(stream)
                    p_tiles = [None] * len(stream)
                    op_tiles = {}

                    def emit_st(gidx):
                        j, g, first, last = stream[gidx]
                        sc = psB.tile([128, GW], f32, tag="sc", bufs=2)
                        sc_tiles[gidx] = sc
                        for (so, c0, w, goff) in g["st"]:
                            slot = base + so
                            nc.tensor.matmul(
                                sc[:, goff : goff + w],
                                lhsT=kT[:, slot * 128 : (slot + 1) * 128],
                                rhs=qT[:, j * 512 + c0 : j * 512 + c0 + w],
                                start=True, stop=True)
                        for (goff, span_c) in g["masks"]:
                            nc.vector.tensor_add(
                                out=sc[:, goff : goff + 128],
                                in0=sc[:, goff : goff + 128],
                                in1=mask_sb[:, mi, :])

                    def emit_exp(gidx):
                        j, g, first, last = stream[gidx]
                        p = ppool.tile([128, GW], fp8, tag="p")
                        p_tiles[gidx] = p
                        gw_ = g["width"]
                        # bias -2: shift-invariant for softmax; keeps exp in
                        # fp8e4m3 range (overflow would make NaN, e4m3 has no inf)
                        nc.scalar.activation(out=p[:, 0:gw_], in_=sc_tiles[gidx][:, 0:gw_],
                                             func=mybir.ActivationFunctionType.Exp,
                                             scale=0.125, bias=expb)

                    def emit_pv(gidx):
                        j, g, first, last = stream[gidx]
                        p = p_tiles[gidx]
                        if first:
                            op_tiles[j] = psB.tile([65, 512], f32, tag="oacc",
                                                   bufs=2, name=f"op_j{j}")
                        op = op_tiles[j]
                        if first and g["pv_rem"]:
                            # j=0: span opens on the straddle group. Emit u0
                            # full-width plain (single start=True zeroing the
                            # whole [0:512)), then u1 plain accumulate.
                            (so, goff, oc0, w) = g["pv_pairs"][0]
                            slot = base + so
                            nc.tensor.matmul(
                                op[:, 0:512],
                                lhsT=v_sb[:, slot, 0 : H + 1],
                                rhs=p[:, 0:512],
                                start=True, stop=False)
                            nc.tensor.matmul(
                                op[:, 128:512],
                                lhsT=v_sb[:, slot + 1, 0 : H + 1],
                                rhs=p[:, 512:896],
                                start=False, stop=last)
                            return
                        n_instr = len(g["pv_pairs"]) + len(g["pv_rem"])
                        k = 0
                        for (so, goff, oc0, w) in g["pv_pairs"]:
                            slot = base + so
                            k += 1
                            if use_pair:
                                nc.tensor.matmul(
                                    op[:, oc0 : oc0 + w],
                                    lhsT=v_sb[:, slot : slot + 2, 0 : H + 1],
                                    rhs=p[:, goff : goff + 2 * w].rearrange("p (t w) -> p t w", t=2),
                                    start=first, stop=(last and k == n_instr),
                                    perf_mode=DR)
                            else:
                                nc.tensor.matmul(
                                    op[:, oc0 : oc0 + w],
                                    lhsT=v_sb[:, slot, 0 : H + 1],
                                    rhs=p[:, goff : goff + w],
                                    start=first, stop=False)
                                nc.tensor.matmul(
                                    op[:, oc0 : oc0 + w],
                                    lhsT=v_sb[:, slot + 1, 0 : H + 1],
                                    rhs=p[:, goff + w : goff + 2 * w],
                                    start=False, stop=(last and k == n_instr))
                        for (so, goff, oc0, w) in g["pv_rem"]:
                            slot = base + so
                            k += 1
                            nc.tensor.matmul(
                                op[:, oc0 : oc0 + w],
                                lhsT=v_sb[:, slot, 0 : H + 1],
                                rhs=p[:, goff : goff + w],
                                start=first, stop=(last and k == n_instr))
                        if last:
                            if not partner:
                                nc.vector.tensor_copy(out=o_own[:, j, :], in_=op)
                            else:
                                epilogue(j, op)

                    n = len(stream)
                    emit_st(0)
                    emit_exp(0)
                    for gidx in range(n):
                        if gidx + 1 < n:
                            emit_st(gidx + 1)
                            emit_exp(gidx + 1)
                        emit_pv(gidx)

                emit_phase(False, None)

                # ---- pass B (chunk-major, 1-bank accumulator) ----
                with tc.tile_pool(name="psKV", bufs=1, space="PSUM") as psKV:
                    for tck in range(4):
                        kv = psKV.tile([128, 512], f32, tag="kv", bufs=1)
                        for d in range(ND):
                            nc.tensor.matmul(kv, lhsT=wkv_sb[:, d, :],
                                             rhs=xb[d][:, tck * 512 : (tck + 1) * 512],
                                             start=(d == 0), stop=(d == ND - 1))
                        sl = slice(tck * 512, (tck + 1) * 512)
                        nc.vector.tensor_copy(
                            out=kT[:, T // 2 + tck * 512 : T // 2 + (tck + 1) * 512],
                            in_=kv[0:64, :])
                        nc.vector.tensor_copy(out=vT_oth[:, sl], in_=kv[64:128, :])

                # v_oth^T -> natural fp8 layout (slots 16..31)
                for i in range(NOWN):
                    tp = psB.tile([128, 65], bf16, tag="otp", bufs=1)
                    nc.tensor.transpose(tp[:, 0:H], vT_oth[:, i * 128 : (i + 1) * 128],
                                        ident_b[0:64, 0:64])
                    nc.vector.tensor_copy(out=v_sb[:, NOWN + i, 0:H], in_=tp[:, 0:H])

                def epilogue(j, op):
                    o_sb = opool.tile([65, 512], bf16, tag="o")
                    nc.vector.tensor_add(out=o_sb, in0=op, in1=o_own[:, j, :])
                    for u in range(4):
                        tp = psB.tile([128, 65], bf16, tag="otp", bufs=1)
                        nc.tensor.transpose(tp, o_sb[:, u * 128 : (u + 1) * 128],
                                            ident_b[0:65, 0:65])
                        r_sb = opool.tile([128, 1], f32, tag="r", bufs=2)
                        nc.vector.reciprocal(r_sb, tp[:, H : H + 1])
                        nc.vector.tensor_scalar_mul(
                            out=out_sb[:, 4 * j + u, :], in0=tp[:, 0:H], scalar1=r_sb)

                emit_phase(True, epilogue)

            nc.sync.dma_start(out=out.rearrange("p (c h) -> p c h", c=NOWN), in_=out_sb)
    _split_multi_waits(nc)
    return nc


def _host_inputs(x, Wk, Wq, Wv):
    """Build the 8 per-core input maps (partition-major constant layouts)."""
    import ml_dtypes

    bf = ml_dtypes.bfloat16
    ND = D // 128
    maps = []
    wqk = np.concatenate([Wq, Wk], axis=1).reshape(ND, 128, 128).transpose(1, 0, 2)
    wqk = np.ascontiguousarray(wqk.reshape(128, ND * 128)).astype(bf)
    wkv = np.concatenate([Wk, Wv], axis=1).reshape(ND, 128, 128).transpose(1, 0, 2)
    wkv = np.ascontiguousarray(wkv.reshape(128, ND * 128)).astype(bf)
    wvm = Wv.reshape(ND, 128, H).transpose(1, 0, 2)
    wvm = np.ascontiguousarray(wvm.reshape(128, ND * H)).astype(bf)
    s = np.arange(128)[:, None]
    t = np.arange(128)[None, :]
    tri = np.where(s <= t, 0.0, -1e4).astype(np.float32)  # additive causal mask
    xbf = x.astype(bf)
    for c in range(8):
        b, p = c // 2, c % 2
        own = [2 * i + p for i in range(NOWN)]
        oth = [2 * i + (1 - p) for i in range(NOWN)]
        own_rows = np.concatenate([np.arange(g * 128, (g + 1) * 128) for g in own])
        oth_rows = np.concatenate([np.arange(g * 128, (g + 1) * 128) for g in oth])
        xb_ = xbf[b]
        xTc = np.ascontiguousarray(
            np.concatenate([xb_[own_rows].T, xb_[oth_rows].T], axis=1))
        ob = np.full((128, 128), 0.0 if p == 1 else -1e4, np.float32)
        mk = np.stack([tri, ob], axis=1).reshape(128, 2 * 128)
        maps.append({"xT": xTc, "wqk": wqk, "wkv": wkv, "wv": wvm, "masks": mk})
    return maps


def kernel(x, Wk, Wq, Wv):
    global _PROG, LAST_EXEC_TIME_NS, LAST_RESULTS
    from concourse.bass_utils import run_bass_kernel_spmd

    if _PROG is None:
        _PROG = _build_program()
    in_maps = _host_inputs(np.asarray(x, np.float32), np.asarray(Wk, np.float32),
                           np.asarray(Wq, np.float32), np.asarray(Wv, np.float32))
    trace = os.environ.get("BASS_KERNEL_TRACE", "0") == "1"
    res = run_bass_kernel_spmd(_PROG, in_maps, list(range(8)), trace=trace)
    LAST_EXEC_TIME_NS = res.exec_time_ns
    LAST_RESULTS = res
    out = np.zeros((B, T, H), np.float32)
    for c in range(8):
        b, p = c // 2, c % 2
        oc = res.results[c]["out"].reshape(128, NOWN, H)
        for i in range(NOWN):
            g = 2 * i + p
            out[b, g * 128 : (g + 1) * 128] = oc[:, i, :]
    return out
